# revision 2
# baseline (speedup 1.0000x reference)
"""Trainium2 kernel for nn_Custom_Model_Embedding_Bag_Sum.

Math: the reference sums the per-bag EmbeddingBag outputs over ALL bags, and
since offsets[0] == 0 every index position lands in exactly one bag, so the
output reduces to

    s_t[d] = sum_i W[t, eb_input[i], d]  =  sum_v c[v] * W[t, v, d]

with c = histogram(eb_input) over the 1M vocab (exact small-integer counts).
That turns a 40M-row irregular gather into a dense weighted reduction that
reads each table row exactly once, contiguously (memory-roofline-friendly).

Distribution: vocab-sharded across the 8 cores.  Each core gets
  w: [128, 30*Q]  (plane-major: plane i = (table t, dim d); free index q)
  c: [128, Q]     (the matching histogram shard)
and computes red[p, i] = sum_q w[p, i*Q+q] * c[p, q] with the fused
scalar_tensor_tensor (out=(w*1.0)*c, accum_out=sum) on the vector engine,
streaming w in DMA groups overlapped with compute.  Each core returns
red [128, 30] f32; the host sums partitions+cores (tiny) and assembles the
26-vector (tables 5 and 6 additionally sum over d).
"""

import os
import sys

import numpy as np

if "/opt/trn_rl_repo" not in sys.path:
    sys.path.insert(0, "/opt/trn_rl_repo")

NUM_TABLES = 10
EMB_DIM = 3
VOCAB = 1_000_000
N_CORES = 8
P = 128
Q = 978                      # free elems per partition per plane (even, 4B-aligned fp16)
V_CORE = P * Q               # 125184 vocab rows per core
N_PLANES = NUM_TABLES * EMB_DIM  # 30

# knobs
DT = os.environ.get("EBAG_DT", "float32")       # float32 | float16
PG = int(os.environ.get("EBAG_PG", "0"))        # planes per DMA group (0 = auto)

_COMPILED = {}


def _build_nc(np_dt, pg):
    import concourse.bass as bass
    from concourse import mybir

    dt = {np.dtype(np.float32): mybir.dt.float32,
          np.dtype(np.float16): mybir.dt.float16}[np.dtype(np_dt)]
    f32 = mybir.dt.float32

    ng = N_PLANES // pg
    assert ng * pg == N_PLANES

    nc = bass.Bass()
    w = nc.dram_tensor("w", [P, N_PLANES * Q], dt, kind="ExternalInput")
    c = nc.dram_tensor("c", [P, Q], dt, kind="ExternalInput")
    o = nc.dram_tensor("o", [P, N_PLANES], f32, kind="ExternalOutput")

    with (
        nc.sbuf_tensor([P, N_PLANES * Q], dt) as w_sb,
        nc.sbuf_tensor([P, Q], dt) as c_sb,
        nc.sbuf_tensor([P, Q], dt) as prod,
        nc.sbuf_tensor([P, N_PLANES], f32) as red,
        nc.semaphore() as dma_sem,
        nc.semaphore() as v_sem,
        nc.Block() as block,
    ):
        @block.sync
        def _(sync):
            sync.dma_start(c_sb[:], c[:]).then_inc(dma_sem, 16)
            for g in range(ng):
                sl = slice(g * pg * Q, (g + 1) * pg * Q)
                sync.dma_start(w_sb[:, sl], w[:, sl]).then_inc(dma_sem, 16)
            sync.wait_ge(v_sem, 1)
            sync.dma_start(o[:], red[:]).then_inc(dma_sem, 16)
            sync.wait_ge(dma_sem, 16 * (ng + 2))

        @block.vector
        def _(vector):
            for g in range(ng):
                vector.wait_ge(dma_sem, 16 * (g + 2))
                for j in range(pg):
                    i = g * pg + j
                    inst = vector.scalar_tensor_tensor(
                        prod[:],
                        w_sb[:, i * Q:(i + 1) * Q],
                        1.0,
                        c_sb[:],
                        op0=mybir.AluOpType.mult,
                        op1=mybir.AluOpType.mult,
                        accum_out=red[:, i:i + 1],
                    )
                    if i == N_PLANES - 1:
                        inst.then_inc(v_sem)

    return nc


def _get_nc(np_dt, pg):
    key = (np.dtype(np_dt).name, pg)
    if key not in _COMPILED:
        _COMPILED[key] = _build_nc(np_dt, pg)
    return _COMPILED[key]


def _auto_pg(np_dt):
    if PG:
        return PG
    return 3 if np.dtype(np_dt).itemsize == 4 else 6


def _prep_inputs(eb_input, W, np_dt):
    """Per-core input maps: histogram shard + swizzled table shard."""
    counts = np.bincount(eb_input.astype(np.int64), minlength=VOCAB)
    counts_pad = np.zeros(N_CORES * V_CORE, dtype=np_dt)
    counts_pad[:VOCAB] = counts.astype(np_dt)
    c_sh = counts_pad.reshape(N_CORES, P, Q)

    in_maps = []
    for k in range(N_CORES):
        v0, v1 = k * V_CORE, (k + 1) * V_CORE
        if v1 <= VOCAB:
            wk = W[:, v0:v1, :]
        else:
            wk = np.zeros((NUM_TABLES, V_CORE, EMB_DIM), dtype=W.dtype)
            wk[:, :VOCAB - v0, :] = W[:, v0:, :]
        # [10, V_CORE, 3] -> [10, 128, Q, 3] -> (p, t, d, q) -> [128, 30*Q]
        wk = np.ascontiguousarray(
            wk.reshape(NUM_TABLES, P, Q, EMB_DIM).transpose(1, 0, 3, 2),
            dtype=np_dt,
        ).reshape(P, N_PLANES * Q)
        in_maps.append({"w": wk, "c": np.ascontiguousarray(c_sh[k])})
    return in_maps


def _assemble(partials):
    """partials: [n_cores, 128, 30] f32 per-core per-partition sums -> [26]."""
    S = partials.sum(axis=(0, 1)).reshape(NUM_TABLES, EMB_DIM).astype(np.float32)
    parts = []
    for t in range(NUM_TABLES):
        if t in (5, 6):
            parts.append(S[t].sum(keepdims=True))
        else:
            parts.append(S[t])
    return np.concatenate(parts).astype(np.float32)


def kernel(eb_input, eb_offset, W):
    from concourse.bass_utils import run_bass_kernel_spmd

    np_dt = np.dtype(DT)
    nc = _get_nc(np_dt, _auto_pg(np_dt))
    in_maps = _prep_inputs(np.asarray(eb_input), np.asarray(W), np_dt)
    res = run_bass_kernel_spmd(nc, in_maps, core_ids=list(range(N_CORES)))
    partials = np.stack([r["o"] for r in res.results])
    return _assemble(partials)


# revision 4
# speedup vs baseline: 685.3639x; 685.3639x over previous
"""Trainium2 kernel for nn_Custom_Model_Embedding_Bag_Sum.

Math: the reference sums the per-bag EmbeddingBag outputs over ALL bags, and
since offsets[0] == 0 every index position lands in exactly one bag, so the
output reduces to

    s_t[d] = sum_i W[t, eb_input[i], d]  =  sum_v c[v] * W[t, v, d]

with c = histogram(eb_input) over the 1M vocab (exact small-integer counts).
That turns a 40M-row irregular gather into a dense weighted reduction that
reads each table row exactly once, contiguously (memory-roofline-friendly).

Distribution: vocab-sharded across the 8 cores.  Each core gets
  w: [128, 30*Q]  (plane-major: plane i = (table t, dim d); free index q)
  c: [128, Q]     (the matching histogram shard)
and computes red[p, i] = sum_q w[p, i*Q+q] * c[p, q] with the fused
scalar_tensor_tensor (out=(w*1.0)*c, accum_out=sum) on the vector engine,
streaming w in DMA groups overlapped with compute.  Each core returns
red [128, 30] f32; the host sums partitions+cores (tiny) and assembles the
26-vector (tables 5 and 6 additionally sum over d).
"""

import os
import sys

import numpy as np

if "/opt/trn_rl_repo" not in sys.path:
    sys.path.insert(0, "/opt/trn_rl_repo")

NUM_TABLES = 10
EMB_DIM = 3
VOCAB = 1_000_000
N_CORES = 8
P = 128
Q = 978                      # free elems per partition per plane (even, 4B-aligned fp16)
V_CORE = P * Q               # 125184 vocab rows per core
N_PLANES = NUM_TABLES * EMB_DIM  # 30

# knobs
DT = os.environ.get("EBAG_DT", "float32")       # float32 | float16
PG = int(os.environ.get("EBAG_PG", "0"))        # planes per DMA group (0 = auto)

_COMPILED = {}


def _build_nc(np_dt, pg, reps=1):
    """reps>1 repeats the full stream+compute body (for overhead-cancelling
    timing): same data re-DMA'd and re-reduced, output written once."""
    import concourse.bass as bass
    from concourse import mybir

    dt = {np.dtype(np.float32): mybir.dt.float32,
          np.dtype(np.float16): mybir.dt.float16}[np.dtype(np_dt)]
    f32 = mybir.dt.float32

    ng = N_PLANES // pg
    assert ng * pg == N_PLANES

    nc = bass.Bass()
    w = nc.dram_tensor("w", [P, N_PLANES * Q], dt, kind="ExternalInput")
    c = nc.dram_tensor("c", [P, Q], dt, kind="ExternalInput")
    o = nc.dram_tensor("o", [P, N_PLANES], f32, kind="ExternalOutput")

    with (
        nc.sbuf_tensor([P, N_PLANES * Q], dt) as w_sb,
        nc.sbuf_tensor([P, Q], dt) as c_sb,
        nc.sbuf_tensor([P, Q], dt) as prod,
        nc.sbuf_tensor([P, N_PLANES], f32) as red,
        nc.semaphore() as dma_sem,
        nc.semaphore() as v_sem,
        nc.Block() as block,
    ):
        @block.sync
        def _(sync):
            sync.dma_start(c_sb[:], c[:]).then_inc(dma_sem, 16)
            for r in range(reps):
                for g in range(ng):
                    if r > 0:
                        # WAR: vector must be done with this group's planes
                        # from the previous rep before we overwrite them.
                        sync.wait_ge(v_sem, (r - 1) * ng + g + 1)
                    sl = slice(g * pg * Q, (g + 1) * pg * Q)
                    sync.dma_start(w_sb[:, sl], w[:, sl]).then_inc(dma_sem, 16)
            sync.wait_ge(v_sem, reps * ng)
            sync.dma_start(o[:], red[:]).then_inc(dma_sem, 16)
            sync.wait_ge(dma_sem, 16 * (reps * ng + 2))
            sync.sem_clear(dma_sem)
            sync.sem_clear(v_sem)

        @block.vector
        def _(vector):
            for r in range(reps):
                for g in range(ng):
                    vector.wait_ge(dma_sem, 16 * (r * ng + g + 2))
                    for j in range(pg):
                        i = g * pg + j
                        inst = vector.scalar_tensor_tensor(
                            prod[:],
                            w_sb[:, i * Q:(i + 1) * Q],
                            1.0,
                            c_sb[:],
                            op0=mybir.AluOpType.mult,
                            op1=mybir.AluOpType.mult,
                            accum_out=red[:, i:i + 1],
                        )
                        if j == pg - 1:
                            inst.then_inc(v_sem)

    return nc


def _get_nc(np_dt, pg, reps=1):
    key = (np.dtype(np_dt).name, pg, reps)
    if key not in _COMPILED:
        _COMPILED[key] = _build_nc(np_dt, pg, reps)
    return _COMPILED[key]


def _auto_pg(np_dt):
    if PG:
        return PG
    return 3 if np.dtype(np_dt).itemsize == 4 else 6


def _prep_inputs(eb_input, W, np_dt):
    """Per-core input maps: histogram shard + swizzled table shard."""
    counts = np.bincount(eb_input.astype(np.int64), minlength=VOCAB)
    counts_pad = np.zeros(N_CORES * V_CORE, dtype=np_dt)
    counts_pad[:VOCAB] = counts.astype(np_dt)
    c_sh = counts_pad.reshape(N_CORES, P, Q)

    in_maps = []
    for k in range(N_CORES):
        v0, v1 = k * V_CORE, (k + 1) * V_CORE
        if v1 <= VOCAB:
            wk = W[:, v0:v1, :]
        else:
            wk = np.zeros((NUM_TABLES, V_CORE, EMB_DIM), dtype=W.dtype)
            wk[:, :VOCAB - v0, :] = W[:, v0:, :]
        # [10, V_CORE, 3] -> [10, 128, Q, 3] -> (p, t, d, q) -> [128, 30*Q]
        wk = np.ascontiguousarray(
            wk.reshape(NUM_TABLES, P, Q, EMB_DIM).transpose(1, 0, 3, 2),
            dtype=np_dt,
        ).reshape(P, N_PLANES * Q)
        in_maps.append({"w": wk, "c": np.ascontiguousarray(c_sh[k])})
    return in_maps


def _assemble(partials):
    """partials: [n_cores, 128, 30] f32 per-core per-partition sums -> [26]."""
    S = partials.sum(axis=(0, 1)).reshape(NUM_TABLES, EMB_DIM).astype(np.float32)
    parts = []
    for t in range(NUM_TABLES):
        if t in (5, 6):
            parts.append(S[t].sum(keepdims=True))
        else:
            parts.append(S[t])
    return np.concatenate(parts).astype(np.float32)


def kernel(eb_input, eb_offset, W):
    from concourse.bass_utils import run_bass_kernel_spmd

    np_dt = np.dtype(DT)
    nc = _get_nc(np_dt, _auto_pg(np_dt))
    in_maps = _prep_inputs(np.asarray(eb_input), np.asarray(W), np_dt)
    res = run_bass_kernel_spmd(nc, in_maps, core_ids=list(range(N_CORES)))
    partials = np.stack([r["o"] for r in res.results])
    return _assemble(partials)


# revision 14
# speedup vs baseline: 1620.3678x; 2.3642x over previous
"""Trainium2 kernel for nn_Custom_Model_Embedding_Bag_Sum.

Math: the reference sums the per-bag EmbeddingBag outputs over ALL bags, and
since offsets[0] == 0 every index position lands in exactly one bag, so the
output reduces to

    s_t[d] = sum_i W[t, eb_input[i], d]  =  sum_v c[v] * W[t, v, d]

with c = histogram(eb_input) over the 1M vocab (exact small-integer counts).
That turns a 40M-row irregular gather into a dense weighted reduction that
reads each table row exactly once, contiguously (memory-roofline-friendly).

Distribution: vocab-sharded across the 8 cores.  Each core gets
  w: [128, 30*Q]  (plane-major: plane i = (table t, dim d); free index q)
  c: [128, Q]     (the matching histogram shard)
and computes red[p, i] = sum_q w[p, i*Q+q] * c[p, q] with the fused
scalar_tensor_tensor (out=(w*1.0)*c, accum_out=sum) on the vector engine,
streaming w in DMA groups overlapped with compute.  Each core returns
red [128, 30] f32; the host sums partitions+cores (tiny) and assembles the
26-vector (tables 5 and 6 additionally sum over d).
"""

import os
import sys

import numpy as np

if "/opt/trn_rl_repo" not in sys.path:
    sys.path.insert(0, "/opt/trn_rl_repo")

NUM_TABLES = 10
EMB_DIM = 3
VOCAB = 1_000_000
N_CORES = 8
P = 128
Q = 978                      # free elems per partition per plane (even, 4B-aligned fp16)
V_CORE = P * Q               # 125184 vocab rows per core
N_PLANES = NUM_TABLES * EMB_DIM  # 30

# knobs
# float16 halves the DMA traffic (the memory-bound term); histogram counts
# (<=~30) are exact in fp16, W rounding costs ~2e-4 norm rel err vs f32's 7e-7.
DT = os.environ.get("EBAG_DT", "float16")       # float32 | float16
PG = int(os.environ.get("EBAG_PG", "0"))        # planes per DMA group (0 = auto)
DMA_ENG = os.environ.get("EBAG_DMA", "sync")    # sync | scalar | gpsimd

_COMPILED = {}


def _build_nc(np_dt, pg, reps=1):
    """reps>1 repeats the full stream+compute body (for overhead-cancelling
    timing): same data re-DMA'd and re-reduced, output written once."""
    import concourse.bass as bass
    from concourse import mybir

    dt = {np.dtype(np.float32): mybir.dt.float32,
          np.dtype(np.float16): mybir.dt.float16}[np.dtype(np_dt)]
    f32 = mybir.dt.float32

    ng = N_PLANES // pg
    assert ng * pg == N_PLANES

    nc = bass.Bass()
    w = nc.dram_tensor("w", [P, N_PLANES * Q], dt, kind="ExternalInput")
    c = nc.dram_tensor("c", [P, Q], dt, kind="ExternalInput")
    o = nc.dram_tensor("o", [1, N_PLANES], f32, kind="ExternalOutput")

    with (
        nc.sbuf_tensor([P, N_PLANES * Q], dt) as w_sb,
        nc.sbuf_tensor([P, Q], dt) as c_sb,
        nc.sbuf_tensor([P, Q], dt) as prod,
        nc.sbuf_tensor([P, N_PLANES], f32) as red,
        nc.sbuf_tensor([P, 1], f32) as ones,
        nc.sbuf_tensor([1, N_PLANES], f32) as out_sb,
        nc.psum_tensor([1, N_PLANES], f32) as acc,
        nc.semaphore() as dma_sem,
        nc.semaphore() as v_sem,
        nc.semaphore() as pe_sem,
        nc.Block() as block,
    ):
        def dma_prog(eng):
            eng.dma_start(c_sb[:], c[:]).then_inc(dma_sem, 16)
            for r in range(reps):
                for g in range(ng):
                    if r > 0:
                        # WAR: vector must be done with this group's planes
                        # from the previous rep before we overwrite them.
                        eng.wait_ge(v_sem, (r - 1) * ng + g + 1)
                    sl = slice(g * pg * Q, (g + 1) * pg * Q)
                    eng.dma_start(w_sb[:, sl], w[:, sl]).then_inc(dma_sem, 16)
            eng.wait_ge(pe_sem, 2)
            eng.dma_start(o[:], out_sb[:]).then_inc(dma_sem, 16)
            eng.wait_ge(dma_sem, 16 * (reps * ng + 2))

        if DMA_ENG == "sync":
            @block.sync
            def _(sync):
                dma_prog(sync)
        elif DMA_ENG == "scalar":
            @block.scalar
            def _(scalar):
                dma_prog(scalar)
        else:
            @block.gpsimd
            def _(gpsimd):
                dma_prog(gpsimd)

        @block.vector
        def _(vector):
            vector.memset(ones[:], 1.0)
            for r in range(reps):
                for g in range(ng):
                    vector.wait_ge(dma_sem, 16 * (r * ng + g + 2))
                    for j in range(pg):
                        i = g * pg + j
                        inst = vector.scalar_tensor_tensor(
                            prod[:],
                            w_sb[:, i * Q:(i + 1) * Q],
                            1.0,
                            c_sb[:],
                            op0=mybir.AluOpType.mult,
                            op1=mybir.AluOpType.mult,
                            accum_out=red[:, i:i + 1],
                        )
                        if j == pg - 1:
                            inst.then_inc(v_sem)
            vector.wait_ge(pe_sem, 1)
            vector.tensor_copy(out_sb[:], acc[:]).then_inc(pe_sem)

        @block.tensor
        def _(tensor):
            # partition-axis reduction: [128, 30] -> [1, 30]
            tensor.wait_ge(v_sem, reps * ng)
            tensor.matmul(acc[:], ones[:], red[:], start=True, stop=True).then_inc(
                pe_sem
            )

        # Block exit emits an all-engine barrier; the reset epilogue below
        # runs with every engine quiescent (mirrors Tile's reset-drain) so
        # the NEFF can be re-executed from clean semaphore/DGE state.

    assert v_sem.num == dma_sem.num + 1
    assert pe_sem.num == v_sem.num + 1
    nc.sync.drain(semaphore_range=range(dma_sem.num, pe_sem.num + 1))
    nc.sync.sem_clear(dma_sem)
    nc.sync.sem_clear(v_sem)
    nc.sync.sem_clear(pe_sem)
    return nc


def _get_nc(np_dt, pg, reps=1):
    key = (np.dtype(np_dt).name, pg, reps)
    if key not in _COMPILED:
        _COMPILED[key] = _build_nc(np_dt, pg, reps)
    return _COMPILED[key]


def _auto_pg(np_dt):
    if PG:
        return PG
    return 3 if np.dtype(np_dt).itemsize == 4 else 6


def _prep_inputs(eb_input, W, np_dt):
    """Per-core input maps: histogram shard + swizzled table shard."""
    counts = np.bincount(eb_input.astype(np.int64), minlength=VOCAB)
    counts_pad = np.zeros(N_CORES * V_CORE, dtype=np_dt)
    counts_pad[:VOCAB] = counts.astype(np_dt)
    c_sh = counts_pad.reshape(N_CORES, P, Q)

    in_maps = []
    for k in range(N_CORES):
        v0, v1 = k * V_CORE, (k + 1) * V_CORE
        if v1 <= VOCAB:
            wk = W[:, v0:v1, :]
        else:
            wk = np.zeros((NUM_TABLES, V_CORE, EMB_DIM), dtype=W.dtype)
            wk[:, :VOCAB - v0, :] = W[:, v0:, :]
        # [10, V_CORE, 3] -> [10, 128, Q, 3] -> (p, t, d, q) -> [128, 30*Q]
        wk = np.ascontiguousarray(
            wk.reshape(NUM_TABLES, P, Q, EMB_DIM).transpose(1, 0, 3, 2),
            dtype=np_dt,
        ).reshape(P, N_PLANES * Q)
        in_maps.append({"w": wk, "c": np.ascontiguousarray(c_sh[k])})
    return in_maps


def _assemble(partials):
    """partials: [n_cores, 128, 30] f32 per-core per-partition sums -> [26]."""
    S = partials.sum(axis=(0, 1)).reshape(NUM_TABLES, EMB_DIM).astype(np.float32)
    parts = []
    for t in range(NUM_TABLES):
        if t in (5, 6):
            parts.append(S[t].sum(keepdims=True))
        else:
            parts.append(S[t])
    return np.concatenate(parts).astype(np.float32)


def kernel(eb_input, eb_offset, W):
    from concourse.bass_utils import run_bass_kernel_spmd

    np_dt = np.dtype(DT)
    nc = _get_nc(np_dt, _auto_pg(np_dt))
    in_maps = _prep_inputs(np.asarray(eb_input), np.asarray(W), np_dt)
    res = run_bass_kernel_spmd(nc, in_maps, core_ids=list(range(N_CORES)))
    partials = np.stack([r["o"] for r in res.results])
    return _assemble(partials)
